# revision 1
# baseline (speedup 1.0000x reference)
"""Fused tensor-parallel transformer layer for Trainium2 (8 NeuronCores).

Sharding: Megatron-style tensor parallel. Each core owns 4 heads of the
attention block (q/k/v projection rows, o_proj columns) and 1/8 of the FFN
hidden dim (w1 rows, w2 columns). LayerNorms are computed replicated on
every core. One on-device AllReduce joins the attention block to the FFN
block; the final residual sum is assembled on the host from per-core
partial outputs (each core adds x2/8 so the partials sum to the answer).

All activations live transposed ([hid, seq]) so every matmul contracts
over the partition dim with zero on-device transposes. Matmuls run in
float32r (fp32 operands truncated to ~fp22 inside the PE) which is
full-rate on TRN2 for moving dims >= 256.
"""

import math
import ml_dtypes
import numpy as np

import concourse.bass as bass
import concourse.mybir as mybir
import concourse.tile as tile
from concourse import bacc
from concourse.bass_utils import run_bass_kernel_spmd
from concourse.masks import make_identity

FP = mybir.dt.float32
BF = mybir.dt.bfloat16
P = 128
EPS = 1e-6
AF = mybir.ActivationFunctionType
ALU = mybir.AluOpType


def fr(ap):
    return ap.bitcast(mybir.dt.float32r)


CFG_FULL = dict(
    seq=2048, hid=4096, ffn=16384, n_cores=8, n_heads=32,
    d_nope=128, d_rope=64, d_v=128, sb=512, fb=1024,
)


def build_layer_kernel(cfg, mask_mode, ln1_affine, ln2_affine):
    """mask_mode: 'causal' (skip tiles above diag, add mask on diag tiles),
    'zero' (no mask at all), 'full' (add mask everywhere)."""
    seq, hid, ffn = cfg["seq"], cfg["hid"], cfg["ffn"]
    n_cores, n_heads = cfg["n_cores"], cfg["n_heads"]
    d_nope, d_rope, d_v = cfg["d_nope"], cfg["d_rope"], cfg["d_v"]
    SB, FB = cfg["sb"], cfg["fb"]
    half = d_rope // 2
    hpc = n_heads // n_cores              # heads per core
    nkt = hid // P                        # hid k-tiles
    nsb = seq // SB                       # attention s-blocks
    sbt = SB // P                         # sk tiles per s-block
    nskt = seq // P                       # total sk tiles
    n_rope_ot = hpc * d_rope // P         # rope o-tiles (2 heads each)
    qo = hpc * d_nope // P + n_rope_ot    # q/k o-tiles per core
    dvc = hpc * d_v                       # v cols per core
    ndvt = dvc // P                       # o_proj contraction tiles
    fpc = ffn // n_cores                  # ffn rows per core
    nft = fpc // P                        # f tiles per core
    nfb = seq // FB                       # ffn s-blocks
    nsub = FB // SB                       # 512-wide sub blocks per ffn block
    assert hpc % 2 == 0 and half == 32 and d_nope == P and d_v == P

    nc = bacc.Bacc(None, target_bir_lowering=False)

    xt_d = nc.dram_tensor("xt", [hid, seq], FP, kind="ExternalInput")
    wq_d = nc.dram_tensor("wq_t", [nkt, qo, P, P], FP, kind="ExternalInput")
    wk_d = nc.dram_tensor("wk_t", [nkt, qo, P, P], FP, kind="ExternalInput")
    wv_d = nc.dram_tensor("wv_t", [nkt, P, dvc], FP, kind="ExternalInput")
    wo_d = nc.dram_tensor("wo_t", [ndvt, nkt, P, P], FP, kind="ExternalInput")
    w1_d = nc.dram_tensor("w1_t", [nkt, nft, P, P], FP, kind="ExternalInput")
    w2_d = nc.dram_tensor("w2_t", [nft, nkt, P, P], BF, kind="ExternalInput")
    cos_d = nc.dram_tensor("cos_t", [P, seq], FP, kind="ExternalInput")
    sin_d = nc.dram_tensor("sin_t", [P, seq], FP, kind="ExternalInput")
    rp_d = nc.dram_tensor("rperm", [P, P], FP, kind="ExternalInput")
    ones_d = nc.dram_tensor("ones_d", [P, P], FP, kind="ExternalInput")
    ident_d = nc.dram_tensor("ident_d", [P, P], FP, kind="ExternalInput")
    if mask_mode == "causal":
        mask_d = nc.dram_tensor("mask_t", [nsb, sbt, P, SB], FP, kind="ExternalInput")
    elif mask_mode == "full":
        mask_d = nc.dram_tensor("mask_t", [nskt, nsb, P, SB], FP, kind="ExternalInput")
    else:
        mask_d = None
    ln1_d = nc.dram_tensor("ln1_wb", [P, 2 * nkt], FP, kind="ExternalInput") if ln1_affine else None
    ln2_d = nc.dram_tensor("ln2_wb", [P, 2 * nkt], FP, kind="ExternalInput") if ln2_affine else None

    kt_dram = nc.dram_tensor("ktd", [qo, P, seq], FP)
    v_dram = nc.dram_tensor("vd", [nskt, P, dvc], FP)
    ar_in = nc.dram_tensor("ar_in", [nsb, hid, SB], FP)
    ar_out = nc.dram_tensor("ar_out", [nsb, hid, SB], FP)
    x2t_d = nc.dram_tensor("x2t", [hid, seq], FP)
    y_d = nc.dram_tensor("y_t", [hid, seq], FP, kind="ExternalOutput")

    q_nope_scale = 1.0 / math.sqrt(d_nope)
    q_rope_scale = 1.0 / math.sqrt(d_rope)

    _lp = nc.allow_low_precision(
        reason="float32r SBUF views are bit-identical fp32; PE truncates on read")
    _lp.__enter__()
    with tile.TileContext(nc) as tc:
        with (
            tc.tile_pool(name="const", bufs=1) as constp,
            tc.tile_pool(name="stat2", bufs=1) as stat2p,
            tc.tile_pool(name="psmm", bufs=6, space="PSUM") as psmm,
        ):
            ones_col = constp.tile([P, 1], FP)
            nc.sync.dma_start(fr(ones_col[:]), fr(ones_d[:, 0:1]))
            ones_row = constp.tile([1, P], FP)
            nc.sync.dma_start(fr(ones_row[:]), fr(ones_d[0:1, :]))
            eps_t = constp.tile([1, 1], FP)
            nc.any.memset(eps_t[:], EPS)
            rperm_t = constp.tile([P, P], FP)
            nc.sync.dma_start(fr(rperm_t[:]), fr(rp_d[:, :]))
            if mask_d is not None:
                ident = constp.tile([P, P], FP)
                nc.sync.dma_start(fr(ident[:]), fr(ident_d[:, :]))
            if ln1_affine:
                ln1_wb = constp.tile([P, 2 * nkt], FP)
                nc.sync.dma_start(ln1_wb[:], ln1_d[:, :])
            if ln2_affine:
                ln2_wb = constp.tile([P, 2 * nkt], FP)
                nc.sync.dma_start(ln2_wb[:], ln2_d[:, :])
            a2_all = stat2p.tile([1, seq], FP, tag="a2")
            c2_all = stat2p.tile([1, seq], FP, tag="c2")

            def bcast(row_sbuf):
                """[1, n<=SB] sbuf -> [P, n] psum via PE rank-1 matmul."""
                n = row_sbuf.shape[-1]
                ps = psmm.tile([P, SB], FP, tag="mm", name="bc")
                ps = ps[:, :n]
                nc.tensor.matmul(ps, fr(ones_row[:]), fr(row_sbuf), start=True, stop=True)
                return ps

            # ---------------- attention block ----------------
            with (
                tc.tile_pool(name="ht", bufs=1) as htp,
                tc.tile_pool(name="stats", bufs=2) as statp,
                tc.tile_pool(name="qt", bufs=1) as qtp,
                tc.tile_pool(name="expp", bufs=3) as expp,
                tc.tile_pool(name="ctxp", bufs=1) as ctxp,
                tc.tile_pool(name="wstr", bufs=4) as wsp,
                tc.tile_pool(name="trig", bufs=1) as trigp,
                tc.tile_pool(name="sqp", bufs=2) as sqp,
                tc.tile_pool(name="maskp", bufs=2) as mp,
                tc.tile_pool(name="miscp", bufs=2) as miscp,
                tc.tile_pool(name="psln", bufs=1, space="PSUM") as psln,
            ):
                def ln_stats(sum_ps, ssq_ps, sb_sl):
                    mu = statp.tile([1, SB], FP, tag="mu")
                    nc.scalar.activation(mu[:], sum_ps[:1, :], AF.Copy, scale=1.0 / hid)
                    msq = statp.tile([1, SB], FP, tag="msq")
                    nc.scalar.activation(msq[:], ssq_ps[:1, :], AF.Copy, scale=1.0 / hid)
                    var = statp.tile([1, SB], FP, tag="var")
                    nc.vector.tensor_tensor(var[:], mu[:], mu[:], ALU.mult)
                    nc.vector.tensor_tensor(var[:], msq[:], var[:], ALU.subtract)
                    std = statp.tile([1, SB], FP, tag="std")
                    nc.scalar.activation(std[:], var[:], AF.Sqrt, bias=eps_t[:])
                    rstd = statp.tile([1, SB], FP, tag="rstd")
                    nc.vector.reciprocal(fr(rstd[:]), std[:])
                    nmr = statp.tile([1, SB], FP, tag="nmr")
                    nc.vector.tensor_tensor(fr(nmr[:]), mu[:], rstd[:], ALU.mult)
                    nc.vector.tensor_scalar_mul(fr(nmr[:]), nmr[:], -1.0)
                    return rstd, nmr

                def rope_apply(dest, raw, cs, sn):
                    """dest/raw: [P, SB]; rows (per 64-pair): x1 | x2.
                    out = raw*cos + swap(raw)*sin_signed, swap via PE perm."""
                    ps_sw = psmm.tile([P, SB], FP, tag="mm", name="swp")
                    nc.tensor.matmul(ps_sw[:], fr(rperm_t[:]), fr(raw[:]),
                                     start=True, stop=True)
                    m1 = miscp.tile([P, SB], FP, tag="m1")
                    m2 = miscp.tile([P, SB], FP, tag="m2")
                    nc.vector.tensor_tensor(m1[:], raw[:], cs, ALU.mult)
                    nc.vector.tensor_tensor(m2[:], ps_sw[:], sn, ALU.mult)
                    nc.vector.tensor_tensor(fr(dest), m1[:], m2[:], ALU.add)

                for sb in range(nsb):
                    ssl = slice(sb * SB, (sb + 1) * SB)
                    # --- stage A: LN1 into ht (in place over the x tiles) ---
                    ht = htp.tile([P, nkt, SB], FP, tag="ht")
                    sum_ps = psln.tile([1, SB], FP, tag="lsum")
                    ssq_ps = psln.tile([1, SB], FP, tag="lssq")
                    for kt in range(nkt):
                        nc.sync.dma_start(fr(ht[:, kt, :]), fr(xt_d[kt * P:(kt + 1) * P, ssl]))
                        sq = sqp.tile([P, SB], FP, tag="sq")
                        nc.vector.tensor_tensor(fr(sq[:]), ht[:, kt, :], ht[:, kt, :], ALU.mult)
                        nc.tensor.matmul(sum_ps[:], fr(ones_col[:]), fr(ht[:, kt, :]),
                                         start=(kt == 0), stop=(kt == nkt - 1))
                        nc.tensor.matmul(ssq_ps[:], fr(ones_col[:]), fr(sq[:]),
                                         start=(kt == 0), stop=(kt == nkt - 1))
                    rstd, nmr = ln_stats(sum_ps, ssq_ps, ssl)
                    ab_ps = bcast(rstd[:])
                    cb_ps = bcast(nmr[:])
                    for kt in range(nkt):
                        nc.vector.tensor_tensor(fr(ht[:, kt, :]), ht[:, kt, :], ab_ps, ALU.mult)
                        nc.vector.tensor_tensor(fr(ht[:, kt, :]), ht[:, kt, :], cb_ps, ALU.add)
                        if ln1_affine:
                            nc.vector.tensor_scalar(
                                fr(ht[:, kt, :]), ht[:, kt, :],
                                ln1_wb[:, kt:kt + 1], ln1_wb[:, nkt + kt:nkt + kt + 1],
                                ALU.mult, ALU.add)

                    # --- stage B: q/k/v projections for this s-block ---
                    cs_t = trigp.tile([P, SB], FP, tag="cos")
                    sn_t = trigp.tile([P, SB], FP, tag="sin")
                    nc.sync.dma_start(cs_t[:], cos_d[:, ssl])
                    nc.sync.dma_start(sn_t[:], sin_d[:, ssl])
                    qt = qtp.tile([P, qo, SB], FP, tag="qt")
                    for which, w_d in (("q", wq_d), ("k", wk_d)):
                        for ot in range(qo):
                            mm_ps = psmm.tile([P, SB], FP, tag="mm")
                            for kt in range(nkt):
                                wch = wsp.tile([P, P], FP, tag="w")
                                nc.sync.dma_start(fr(wch[:]), fr(w_d[kt, ot]))
                                nc.tensor.matmul(mm_ps[:], fr(wch[:]), fr(ht[:, kt, :]),
                                                 start=(kt == 0), stop=(kt == nkt - 1))
                            is_rope = ot >= qo - n_rope_ot
                            if which == "q":
                                scale = q_rope_scale if is_rope else q_nope_scale
                                dest = qt[:, ot, :]
                            else:
                                scale = 1.0
                                stg = miscp.tile([P, SB], FP, tag="kvst")
                                dest = stg[:]
                            if not is_rope:
                                nc.scalar.activation(fr(dest), mm_ps[:], AF.Copy, scale=scale)
                            else:
                                raw = miscp.tile([P, SB], FP, tag="raw")
                                nc.scalar.activation(fr(raw[:]), mm_ps[:], AF.Copy, scale=scale)
                                rope_apply(dest, raw, cs_t[:], sn_t[:])
                            if which == "k":
                                nc.sync.dma_start(kt_dram[ot, :, ssl], stg[:])
                    v_pss = [psmm.tile([P, dvc], FP, tag="mm", name=f"vps{_i}") for _i in range(sbt)]
                    for kt in range(nkt):
                        wvch = wsp.tile([P, dvc], FP, tag="wv")
                        nc.sync.dma_start(fr(wvch[:]), fr(wv_d[kt]))
                        for sc in range(sbt):
                            nc.tensor.matmul(
                                v_pss[sc][:], fr(ht[:, kt, sc * P:(sc + 1) * P]), fr(wvch[:]),
                                start=(kt == 0), stop=(kt == nkt - 1))
                    for sc in range(sbt):
                        vst = miscp.tile([P, dvc], FP, tag="kvst")
                        nc.vector.tensor_copy(out=vst[:], in_=v_pss[sc][:])
                        nc.sync.dma_start(v_dram[sb * sbt + sc], vst[:])

                    # --- stage C: attention for q-block sb ---
                    t_max = (sb + 1) * sbt if mask_mode == "causal" else nskt
                    ctxt = ctxp.tile([P, hpc, SB], FP, tag="ctx")
                    for h in range(hpc):
                        rot = qo - n_rope_ot + h // 2
                        rsl = slice(64 * (h % 2), 64 * (h % 2) + 64)
                        sum_ps = psmm.tile([1, SB], FP, tag="mm")
                        ctx_ps = psmm.tile([P, SB], FP, tag="mm")
                        for t in range(t_max):
                            st_ps = psmm.tile([P, SB], FP, tag="mm")
                            tsl = slice(t * P, (t + 1) * P)
                            has_mask = mask_d is not None and (
                                mask_mode == "full" or t >= sb * sbt)
                            kn = wsp.tile([P, P], FP, tag="kl")
                            nc.sync.dma_start(fr(kn[:]), fr(kt_dram[h, :, tsl]))
                            kr = wsp.tile([P, P], FP, tag="krl")
                            nc.sync.dma_start(fr(kr[:]), fr(kt_dram[rot, :, tsl]))
                            vl = wsp.tile([P, P], FP, tag="vl")
                            nc.sync.dma_start(fr(vl[:]), fr(v_dram[t, :, h * P:(h + 1) * P]))
                            nc.tensor.matmul(st_ps[:], fr(kn[:]),
                                             fr(qt[:, h, :]), start=True, stop=False)
                            nc.tensor.matmul(st_ps[:], fr(kr[rsl, :]),
                                             fr(qt[rsl, rot, :]),
                                             start=False, stop=not has_mask)
                            if has_mask:
                                mt = mp.tile([P, SB], FP, tag="mask")
                                if mask_mode == "causal":
                                    nc.sync.dma_start(fr(mt[:]), fr(mask_d[sb, t - sb * sbt]))
                                else:
                                    nc.sync.dma_start(fr(mt[:]), fr(mask_d[t, sb]))
                                nc.tensor.matmul(st_ps[:], fr(ident[:]), fr(mt[:]),
                                                 start=False, stop=True)
                            es = expp.tile([P, SB], FP, tag="es")
                            nc.scalar.activation(fr(es[:]), st_ps[:], AF.Exp)
                            nc.tensor.matmul(sum_ps[:], fr(ones_col[:]), fr(es[:]),
                                             start=(t == 0), stop=(t == t_max - 1))
                            nc.tensor.matmul(ctx_ps[:], fr(vl[:]),
                                             fr(es[:]), start=(t == 0), stop=(t == t_max - 1))
                        rec = statp.tile([1, SB], FP, tag="rec")
                        nc.vector.reciprocal(fr(rec[:]), sum_ps[:1, :])
                        rb_ps = bcast(rec[:])
                        rb = miscp.tile([P, SB], FP, tag="rb")
                        nc.scalar.activation(rb[:], rb_ps[:], AF.Copy)
                        nc.vector.tensor_tensor(fr(ctxt[:, h, :]), ctx_ps[:], rb[:], ALU.mult)

                    # --- stage D: partial o_proj -> ar_in ---
                    for hc in range(nkt):
                        o_ps = psmm.tile([P, SB], FP, tag="mm")
                        for dvt in range(ndvt):
                            wch = wsp.tile([P, P], FP, tag="w")
                            nc.sync.dma_start(fr(wch[:]), fr(wo_d[dvt, hc]))
                            nc.tensor.matmul(o_ps[:], fr(wch[:]), fr(ctxt[:, dvt, :]),
                                             start=(dvt == 0), stop=(dvt == ndvt - 1))
                        ao = miscp.tile([P, SB], FP, tag="m1")
                        nc.vector.tensor_copy(out=ao[:], in_=o_ps[:])
                        nc.sync.dma_start(ar_in[sb, hc * P:(hc + 1) * P, :], ao[:])
                    nc.gpsimd.collective_compute(
                        "AllReduce", ALU.add,
                        replica_groups=[list(range(n_cores))],
                        ins=[ar_in[sb].opt()], outs=[ar_out[sb].opt()])

                # --- stage E: x2 = x + attn_out; LN2 stats; x2t to DRAM ---
                for sb in range(nsb):
                    ssl = slice(sb * SB, (sb + 1) * SB)
                    sum_ps = psln.tile([1, SB], FP, tag="lsum")
                    ssq_ps = psln.tile([1, SB], FP, tag="lssq")
                    for kt in range(nkt):
                        xtile = miscp.tile([P, SB], FP, tag="m2")
                        nc.sync.dma_start(fr(xtile[:]), fr(xt_d[kt * P:(kt + 1) * P, ssl]))
                        artile = miscp.tile([P, SB], FP, tag="raw")
                        nc.sync.dma_start(artile[:], ar_out[sb, kt * P:(kt + 1) * P, :])
                        nc.vector.tensor_tensor(fr(xtile[:]), xtile[:], artile[:], ALU.add)
                        nc.sync.dma_start(x2t_d[kt * P:(kt + 1) * P, ssl], xtile[:])
                        sq = sqp.tile([P, SB], FP, tag="sq")
                        nc.vector.tensor_tensor(fr(sq[:]), xtile[:], xtile[:], ALU.mult)
                        nc.tensor.matmul(sum_ps[:], fr(ones_col[:]), fr(xtile[:]),
                                         start=(kt == 0), stop=(kt == nkt - 1))
                        nc.tensor.matmul(ssq_ps[:], fr(ones_col[:]), fr(sq[:]),
                                         start=(kt == 0), stop=(kt == nkt - 1))
                    rstd, nmr = ln_stats(sum_ps, ssq_ps, ssl)
                    nc.vector.tensor_copy(out=fr(a2_all[:, ssl]), in_=rstd[:])
                    nc.vector.tensor_copy(out=fr(c2_all[:, ssl]), in_=nmr[:])

            # ---------------- FFN block ----------------
            with (
                tc.tile_pool(name="h2p", bufs=1) as h2p,
                tc.tile_pool(name="utp", bufs=1) as utp,
                tc.tile_pool(name="wfp", bufs=4) as wfp,
                tc.tile_pool(name="x2sp", bufs=2) as x2sp,
            ):
                for fb in range(nfb):
                    fsl = slice(fb * FB, (fb + 1) * FB)
                    h2 = h2p.tile([P, nkt, FB], FP, tag="h2")
                    ab_pss, cb_pss = [], []
                    for sub in range(nsub):
                        st = slice(fb * FB + sub * SB, fb * FB + (sub + 1) * SB)
                        ab_pss.append(bcast(a2_all[:, st]))
                        cb_pss.append(bcast(c2_all[:, st]))
                    for kt in range(nkt):
                        for sub in range(nsub):
                            dsl = slice(sub * SB, (sub + 1) * SB)
                            st = slice(fb * FB + sub * SB, fb * FB + (sub + 1) * SB)
                            x2tile = x2sp.tile([P, SB], FP, tag="x2l")
                            nc.sync.dma_start(x2tile[:], x2t_d[kt * P:(kt + 1) * P, st])
                            nc.vector.tensor_tensor(fr(h2[:, kt, dsl]), x2tile[:], ab_pss[sub], ALU.mult)
                            nc.vector.tensor_tensor(fr(h2[:, kt, dsl]), h2[:, kt, dsl], cb_pss[sub], ALU.add)
                            if ln2_affine:
                                nc.vector.tensor_scalar(
                                    fr(h2[:, kt, dsl]), h2[:, kt, dsl],
                                    ln2_wb[:, kt:kt + 1], ln2_wb[:, nkt + kt:nkt + kt + 1],
                                    ALU.mult, ALU.add)
                    ut = utp.tile([P, nft, FB], BF, tag="ut")
                    for ft in range(nft):
                        u_pss = [psmm.tile([P, SB], FP, tag="mm", name=f"ups{_i}") for _i in range(nsub)]
                        for kt in range(nkt):
                            wch = wfp.tile([P, P], FP, tag="w1")
                            nc.sync.dma_start(fr(wch[:]), fr(w1_d[kt, ft]))
                            for sub in range(nsub):
                                nc.tensor.matmul(
                                    u_pss[sub][:], fr(wch[:]),
                                    fr(h2[:, kt, sub * SB:(sub + 1) * SB]),
                                    start=(kt == 0), stop=(kt == nkt - 1))
                        for sub in range(nsub):
                            nc.scalar.activation(ut[:, ft, sub * SB:(sub + 1) * SB],
                                                 u_pss[sub][:], AF.Silu)
                    for hc in range(nkt):
                        y_pss = [psmm.tile([P, SB], FP, tag="mm", name=f"yps{_i}") for _i in range(nsub)]
                        for ft in range(nft):
                            wch = wfp.tile([P, P], BF, tag="w2")
                            nc.sync.dma_start(wch[:], w2_d[ft, hc])
                            for sub in range(nsub):
                                nc.tensor.matmul(
                                    y_pss[sub][:], wch[:],
                                    ut[:, ft, sub * SB:(sub + 1) * SB],
                                    start=(ft == 0), stop=(ft == nft - 1))
                        for sub in range(nsub):
                            st = slice(fb * FB + sub * SB, fb * FB + (sub + 1) * SB)
                            x2tile = x2sp.tile([P, SB], FP, tag="x2r")
                            nc.sync.dma_start(x2tile[:], x2t_d[hc * P:(hc + 1) * P, st])
                            yt = x2sp.tile([P, SB], FP, tag="yt")
                            nc.vector.tensor_scalar_mul(yt[:], x2tile[:], 1.0 / n_cores)
                            nc.vector.tensor_tensor(yt[:], y_pss[sub][:], yt[:], ALU.add)
                            nc.sync.dma_start(y_d[hc * P:(hc + 1) * P, st], yt[:])

    _lp.__exit__(None, None, None)
    nc.compile()
    return nc


# ---------------------------------------------------------------------------
# host side
# ---------------------------------------------------------------------------

def _chunk2d(a, pr, pc):
    """[R, C] -> [R//pr, C//pc, pr, pc] contiguous chunk layout."""
    R, C = a.shape
    return np.ascontiguousarray(
        a.reshape(R // pr, pr, C // pc, pc).transpose(0, 2, 1, 3))


def make_core_inputs(inputs, cfg, mask_mode, ln1_affine, ln2_affine):
    seq, hid, ffn = cfg["seq"], cfg["hid"], cfg["ffn"]
    n_cores, n_heads = cfg["n_cores"], cfg["n_heads"]
    d_nope, d_rope, d_v = cfg["d_nope"], cfg["d_rope"], cfg["d_v"]
    SB = cfg["sb"]
    hpc = n_heads // n_cores
    nkt = hid // P
    nsb = seq // SB
    sbt = SB // P
    nskt = seq // P
    fpc = ffn // n_cores

    f32 = np.float32
    x = np.asarray(inputs["hidden_states"], dtype=f32)[0]        # [seq, hid]
    xt = np.ascontiguousarray(x.T)                                # [hid, seq]

    inv = (1.0 / (10000.0 ** (np.arange(0, d_rope, 2, dtype=f32) / f32(d_rope)))).astype(f32)
    t = np.arange(seq, dtype=f32)
    freqs = t[:, None] * inv[None, :]
    cosT = np.cos(freqs).astype(f32).T                      # [half, seq]
    sinT = np.sin(freqs).astype(f32).T
    cos128 = np.ascontiguousarray(np.tile(cosT, (P // (d_rope // 2), 1)))
    sin128 = np.ascontiguousarray(
        np.tile(np.concatenate([-sinT, sinT], axis=0), (P // d_rope, 1)))
    half = d_rope // 2
    rperm = np.zeros((P, P), dtype=f32)
    for blk in range(P // d_rope):
        b = blk * d_rope
        for i in range(half):
            # out[b+i] takes in[b+half+i]; out[b+half+i] takes in[b+i]
            rperm[b + half + i, b + i] = 1.0
            rperm[b + i, b + half + i] = 1.0

    common = {"xt": xt, "cos_t": cos128, "sin_t": sin128, "rperm": rperm,
              "ones_d": np.ones((P, P), dtype=f32),
              "ident_d": np.eye(P, dtype=f32)}
    mask = np.asarray(inputs["attention_mask"], dtype=f32)[0, 0]  # [seq, seq]
    mT = np.ascontiguousarray(mask.T)                             # [sk, sq]
    if mask_mode == "causal":
        m = np.empty((nsb, sbt, P, SB), dtype=f32)
        for qb in range(nsb):
            for i in range(sbt):
                tt = qb * sbt + i
                m[qb, i] = mT[tt * P:(tt + 1) * P, qb * SB:(qb + 1) * SB]
        common["mask_t"] = m
    elif mask_mode == "full":
        m = np.empty((nskt, nsb, P, SB), dtype=f32)
        for tt in range(nskt):
            for qb in range(nsb):
                m[tt, qb] = mT[tt * P:(tt + 1) * P, qb * SB:(qb + 1) * SB]
        common["mask_t"] = m
    if ln1_affine:
        common["ln1_wb"] = np.ascontiguousarray(np.stack(
            [np.asarray(inputs["ln1_w"], f32), np.asarray(inputs["ln1_b"], f32)]
        ).reshape(2, nkt, P).transpose(2, 0, 1).reshape(P, 2 * nkt))
    if ln2_affine:
        common["ln2_wb"] = np.ascontiguousarray(np.stack(
            [np.asarray(inputs["ln2_w"], f32), np.asarray(inputs["ln2_b"], f32)]
        ).reshape(2, nkt, P).transpose(2, 0, 1).reshape(P, 2 * nkt))

    wq = np.asarray(inputs["w_q"], f32)
    wk = np.asarray(inputs["w_k"], f32)
    wv = np.asarray(inputs["w_v"], f32)
    wo = np.asarray(inputs["w_o"], f32)
    w1 = np.asarray(inputs["w1"], f32)
    w2 = np.asarray(inputs["w2"], f32)

    in_maps = []
    for c in range(n_cores):
        heads = range(c * hpc, (c + 1) * hpc)
        nope = np.concatenate([wq[g * d_nope:(g + 1) * d_nope] for g in heads])
        rope = np.concatenate(
            [wq[n_heads * d_nope + g * d_rope: n_heads * d_nope + (g + 1) * d_rope]
             for g in heads])
        wq_t = _chunk2d(np.concatenate([nope, rope]).T, P, P)
        nope = np.concatenate([wk[g * d_nope:(g + 1) * d_nope] for g in heads])
        rope = np.concatenate(
            [wk[n_heads * d_nope + g * d_rope: n_heads * d_nope + (g + 1) * d_rope]
             for g in heads])
        wk_t = _chunk2d(np.concatenate([nope, rope]).T, P, P)
        wv_c = np.concatenate([wv[g * d_v:(g + 1) * d_v] for g in heads])   # [dvc, hid]
        wv_t = np.ascontiguousarray(wv_c.T.reshape(nkt, P, hpc * d_v))
        wo_c = wo[:, c * hpc * d_v:(c + 1) * hpc * d_v]                      # [hid, dvc]
        wo_t = _chunk2d(np.ascontiguousarray(wo_c.T), P, P)
        w1_t = _chunk2d(np.ascontiguousarray(w1[c * fpc:(c + 1) * fpc].T), P, P)
        w2_t = _chunk2d(np.ascontiguousarray(w2[:, c * fpc:(c + 1) * fpc].T), P, P).astype(ml_dtypes.bfloat16)
        in_maps.append(dict(common, wq_t=wq_t, wk_t=wk_t, wv_t=wv_t, wo_t=wo_t,
                            w1_t=w1_t, w2_t=w2_t))
    return in_maps


def detect_mask_mode(mask, seq):
    if not mask.any():
        return "zero"
    iu = np.triu_indices(seq, 1)
    upper_blocked = bool((mask[iu] <= -1e8).all())
    il = np.tril_indices(seq)
    lower_zero = bool((mask[il] == 0).all())
    if upper_blocked and lower_zero:
        return "causal"
    return "full"


_BUILT = {}


def run_layer(inputs, cfg, trace=False):
    f32 = np.float32
    mask = np.asarray(inputs["attention_mask"], dtype=f32)[0, 0]
    mask_mode = detect_mask_mode(mask, cfg["seq"])
    ln1_affine = not ((np.asarray(inputs["ln1_w"]) == 1).all()
                     and (np.asarray(inputs["ln1_b"]) == 0).all())
    ln2_affine = not ((np.asarray(inputs["ln2_w"]) == 1).all()
                     and (np.asarray(inputs["ln2_b"]) == 0).all())
    key = (tuple(sorted(cfg.items())), mask_mode, ln1_affine, ln2_affine)
    if key not in _BUILT:
        _BUILT[key] = build_layer_kernel(cfg, mask_mode, ln1_affine, ln2_affine)
    nc = _BUILT[key]
    in_maps = make_core_inputs(inputs, cfg, mask_mode, ln1_affine, ln2_affine)
    res = run_bass_kernel_spmd(nc, in_maps, core_ids=list(range(cfg["n_cores"])),
                               trace=trace)
    acc = np.zeros((cfg["hid"], cfg["seq"]), dtype=np.float64)
    for c in range(cfg["n_cores"]):
        acc += res.results[c]["y_t"]
    out = acc.T.astype(f32)[None]
    return out, res


def kernel(**inputs):
    out, _ = run_layer(inputs, CFG_FULL)
    return out



# revision 7
# speedup vs baseline: 1.9770x; 1.9770x over previous
"""Fused tensor-parallel transformer layer for Trainium2 (8 NeuronCores), v2.

Megatron-style TP as before (4 heads + 1/8 FFN per core), but rebuilt around
what the v1 trace showed:
  * fp32 moving operands stream through the PE at half rate -> all matmul
    operands are bf16 now (fp32 PSUM accumulate).
  * per-(h,t) k/v DRAM round-trips stalled the PE and HAM-rethrottled the
    clock -> q/k/v live in SBUF for the whole attention phase.
  * 845k tiny DMA descriptors -> weights are DMAed as [P, nkt*P] slabs.
  * AllReduce in bf16 (half the wire bytes), output buffer addr_space=Shared.
  * causal mask applied as a multiplicative 0/1 tri mask after exp (no mask
    matmul); softmax exp->sum/ctx chain software-pipelined by one k-tile so
    the in-order PE queue never waits on the ScalarE exp.
"""

import math
import ml_dtypes
import numpy as np

import concourse.bass as bass
import concourse.mybir as mybir
import concourse.tile as tile
from concourse import bacc
from concourse.bass_utils import run_bass_kernel_spmd

FP = mybir.dt.float32
BF = mybir.dt.bfloat16
P = 128
EPS = 1e-6
AF = mybir.ActivationFunctionType
ALU = mybir.AluOpType

CFG_FULL = dict(
    seq=2048, hid=4096, ffn=16384, n_cores=8, n_heads=32,
    d_nope=128, d_rope=64, d_v=128, sb=512, fb=1024,
)


def build_layer_kernel(cfg, mask_mode, ln1_affine, ln2_affine):
    seq, hid, ffn = cfg["seq"], cfg["hid"], cfg["ffn"]
    n_cores, n_heads = cfg["n_cores"], cfg["n_heads"]
    d_nope, d_rope, d_v = cfg["d_nope"], cfg["d_rope"], cfg["d_v"]
    SB, FB = cfg["sb"], cfg["fb"]
    half = d_rope // 2
    hpc = n_heads // n_cores              # heads per core
    nkt = hid // P                        # hid k-tiles
    nsb = seq // SB                       # attention s-blocks
    sbt = SB // P                         # sk tiles per s-block
    nskt = seq // P                       # total sk tiles
    n_rope_ot = hpc * d_rope // P         # rope o-tiles (2 heads each)
    qo = hpc * d_nope // P + n_rope_ot    # q/k o-tiles per core
    dvc = hpc * d_v                       # v cols per core
    ndvt = dvc // P                       # o_proj contraction tiles
    fpc = ffn // n_cores                  # ffn rows per core
    nft = fpc // P                        # f tiles per core
    nfb = seq // FB                       # ffn s-blocks
    nsub = FB // SB                       # 512-wide sub blocks per ffn block
    assert hpc % 2 == 0 and half == 32 and d_nope == P and d_v == P

    nc = bacc.Bacc(None, target_bir_lowering=False)

    xt_d = nc.dram_tensor("xt", [hid, seq], FP, kind="ExternalInput")
    wq_d = nc.dram_tensor("wq_t", [qo, P, nkt * P], BF, kind="ExternalInput")
    wk_d = nc.dram_tensor("wk_t", [qo, P, nkt * P], BF, kind="ExternalInput")
    wv_d = nc.dram_tensor("wv_t", [nkt, P, dvc], BF, kind="ExternalInput")
    wo_d = nc.dram_tensor("wo_t", [nkt, P, ndvt * P], BF, kind="ExternalInput")
    w1_d = nc.dram_tensor("w1_t", [nft, P, nkt * P], BF, kind="ExternalInput")
    w2_d = nc.dram_tensor("w2_t", [nkt, P, nft * P], BF, kind="ExternalInput")
    cos_d = nc.dram_tensor("cos_t", [P, seq], FP, kind="ExternalInput")
    sin_d = nc.dram_tensor("sin_t", [P, seq], FP, kind="ExternalInput")
    rp_d = nc.dram_tensor("rperm", [P, P], BF, kind="ExternalInput")
    onb_d = nc.dram_tensor("ones_b", [P, P], BF, kind="ExternalInput")
    onf_d = nc.dram_tensor("ones_f", [P, P], FP, kind="ExternalInput")
    if mask_mode == "causal":
        tri_d = nc.dram_tensor("tri_t", [sbt, P, SB], BF, kind="ExternalInput")
    elif mask_mode == "full":
        mask_d = nc.dram_tensor("mask_t", [nskt, nsb, P, SB], BF, kind="ExternalInput")
        ident_d = nc.dram_tensor("ident_d", [P, P], BF, kind="ExternalInput")
    ln1_d = nc.dram_tensor("ln1_wb", [P, 2 * nkt], FP, kind="ExternalInput") if ln1_affine else None
    ln2_d = nc.dram_tensor("ln2_wb", [P, 2 * nkt], FP, kind="ExternalInput") if ln2_affine else None

    ar_in = nc.dram_tensor("ar_in", [nsb, P, nkt, SB], BF)
    ar_out = nc.dram_tensor("ar_out", [nsb, P, nkt, SB], BF, addr_space="Shared")
    x2t_d = nc.dram_tensor("x2t", [nkt, P, seq], BF)
    y_d = nc.dram_tensor("y_t", [nkt, P, seq], FP, kind="ExternalOutput")

    _lp = nc.allow_low_precision(
        reason="bf16 matmul operands / bf16 activations; fp32 PSUM accumulate")
    _lp.__enter__()
    from contextlib import ExitStack
    with tile.TileContext(nc) as tc:
        with ExitStack() as _stk:
            constp = _stk.enter_context(tc.tile_pool(name="const", bufs=1))
            stat2p = _stk.enter_context(tc.tile_pool(name="stat2", bufs=1))
            psln = _stk.enter_context(tc.tile_pool(name="psln", bufs=1, space="PSUM"))
            ones_col = constp.tile([P, 1], BF)          # bf16 stationary for col sums
            nc.sync.dma_start(ones_col[:], onb_d[:, 0:1])
            ones_row = constp.tile([1, P], FP)          # fp32 stationary for bcast
            nc.sync.dma_start(ones_row[:], onf_d[0:1, :])
            eps_t = constp.tile([1, 1], FP)
            nc.any.memset(eps_t[:], EPS)
            rperm_t = constp.tile([P, P], BF)
            nc.sync.dma_start(rperm_t[:], rp_d[:, :])
            if mask_mode == "causal":
                tri_t = constp.tile([P, sbt, SB], BF)
                for i in range(sbt):
                    nc.sync.dma_start(tri_t[:, i, :], tri_d[i])
            if mask_mode == "full":
                ident = constp.tile([P, P], BF)
                nc.sync.dma_start(ident[:], ident_d[:, :])
            if ln1_affine:
                ln1_wb = constp.tile([P, 2 * nkt], FP)
                nc.sync.dma_start(ln1_wb[:], ln1_d[:, :])
            if ln2_affine:
                ln2_wb = constp.tile([P, 2 * nkt], FP)
                nc.sync.dma_start(ln2_wb[:], ln2_d[:, :])
            a2_all = stat2p.tile([1, seq], FP, tag="a2")
            c2_all = stat2p.tile([1, seq], FP, tag="c2")

            # ---------------- attention block ----------------
            with ExitStack() as _astk:
                cachep = _astk.enter_context(tc.tile_pool(name="cache", bufs=1))
                kcache = cachep.tile([P, qo, seq], BF, tag="kc")
                vcache = cachep.tile([P, nskt, dvc], BF, tag="vc")
                xhtp = _astk.enter_context(tc.tile_pool(name="xht", bufs=1))
                qtp = _astk.enter_context(tc.tile_pool(name="qt", bufs=2))
                ctxp = _astk.enter_context(tc.tile_pool(name="ctx", bufs=2))
                wqkp = _astk.enter_context(tc.tile_pool(name="wqk", bufs=2))
                wsmp = _astk.enter_context(tc.tile_pool(name="wsm", bufs=3))
                xfp = _astk.enter_context(tc.tile_pool(name="xf", bufs=3))
                sqp = _astk.enter_context(tc.tile_pool(name="sq", bufs=2))
                esp = _astk.enter_context(tc.tile_pool(name="es", bufs=4))
                statp = _astk.enter_context(tc.tile_pool(name="stats", bufs=1))
                absp = _astk.enter_context(tc.tile_pool(name="absb", bufs=2))
                miscp = _astk.enter_context(tc.tile_pool(name="misc", bufs=2))
                trigp = _astk.enter_context(tc.tile_pool(name="trig", bufs=2))
                psmm = _astk.enter_context(tc.tile_pool(name="psmm", bufs=4, space="PSUM"))
                psacc = _astk.enter_context(tc.tile_pool(name="psacc", bufs=2, space="PSUM"))
                def ln_stats(sum_ps, ssq_ps):
                    mu = statp.tile([1, SB], FP, tag="mu")
                    nc.scalar.activation(mu[:], sum_ps[:1, :], AF.Copy, scale=1.0 / hid)
                    msq = statp.tile([1, SB], FP, tag="msq")
                    nc.scalar.activation(msq[:], ssq_ps[:1, :], AF.Copy, scale=1.0 / hid)
                    var = statp.tile([1, SB], FP, tag="var")
                    nc.vector.tensor_tensor(var[:], mu[:], mu[:], ALU.mult)
                    nc.vector.tensor_tensor(var[:], msq[:], var[:], ALU.subtract)
                    std = statp.tile([1, SB], FP, tag="std")
                    nc.scalar.activation(std[:], var[:], AF.Sqrt, bias=eps_t[:])
                    rstd = statp.tile([1, SB], FP, tag="rstd")
                    nc.vector.reciprocal(rstd[:], std[:])
                    nmr = statp.tile([1, SB], FP, tag="nmr")
                    nc.vector.tensor_tensor(nmr[:], mu[:], rstd[:], ALU.mult)
                    nc.vector.tensor_scalar_mul(nmr[:], nmr[:], -1.0)
                    return rstd, nmr

                def bcast_sb(row, tag):
                    """[1, SB] fp32 row -> SBUF bf16 [P, SB] via PE rank-1 + copy."""
                    ps = psmm.tile([P, SB], FP, tag="mm", name="bc")
                    nc.tensor.matmul(ps[:], ones_row[:], row, start=True, stop=True)
                    sb_t = absp.tile([P, SB], BF, tag=tag, name=tag)
                    nc.scalar.activation(sb_t[:], ps[:], AF.Copy)
                    return sb_t

                for sb in range(nsb):
                    ssl = slice(sb * SB, (sb + 1) * SB)
                    # --- stage A: LN1 -> xht (bf16, in place) ---
                    xht = xhtp.tile([P, nkt, SB], BF, tag="xht")
                    sum_ps = psln.tile([1, SB], FP, tag="lsum")
                    ssq_ps = psln.tile([1, SB], FP, tag="lssq")
                    for kt in range(nkt):
                        xtile = xfp.tile([P, SB], FP, tag="xf")
                        nc.sync.dma_start(xtile[:], xt_d[kt * P:(kt + 1) * P, ssl])
                        nc.vector.tensor_copy(out=xht[:, kt, :], in_=xtile[:])
                        sq = sqp.tile([P, SB], BF, tag="sq")
                        nc.vector.tensor_tensor(sq[:], xht[:, kt, :], xht[:, kt, :], ALU.mult)
                        nc.tensor.matmul(sum_ps[:], ones_col[:], xht[:, kt, :],
                                         start=(kt == 0), stop=(kt == nkt - 1))
                        nc.tensor.matmul(ssq_ps[:], ones_col[:], sq[:],
                                         start=(kt == 0), stop=(kt == nkt - 1))
                    rstd, nmr = ln_stats(sum_ps, ssq_ps)
                    ab_sb = bcast_sb(rstd[:], "ab")
                    cb_sb = bcast_sb(nmr[:], "cb")
                    for kt in range(nkt):
                        nc.vector.tensor_tensor(xht[:, kt, :], xht[:, kt, :], ab_sb[:], ALU.mult)
                        nc.vector.tensor_tensor(xht[:, kt, :], xht[:, kt, :], cb_sb[:], ALU.add)
                        if ln1_affine:
                            nc.vector.tensor_scalar(
                                xht[:, kt, :], xht[:, kt, :],
                                ln1_wb[:, kt:kt + 1], ln1_wb[:, nkt + kt:nkt + kt + 1],
                                ALU.mult, ALU.add)

                    # --- stage B: q/k/v projections for this s-block ---
                    cs_t = trigp.tile([P, SB], FP, tag="cos")
                    sn_t = trigp.tile([P, SB], FP, tag="sin")
                    nc.sync.dma_start(cs_t[:], cos_d[:, ssl])
                    nc.sync.dma_start(sn_t[:], sin_d[:, ssl])
                    qt = qtp.tile([P, qo, SB], BF, tag="qt")
                    for which, w_d in (("q", wq_d), ("k", wk_d)):
                        for ot in range(qo):
                            wsb = wqkp.tile([P, nkt * P], BF, tag="wqk")
                            nc.sync.dma_start(wsb[:], w_d[ot])
                            mm_ps = psmm.tile([P, SB], FP, tag="mm")
                            for kt in range(nkt):
                                nc.tensor.matmul(mm_ps[:], wsb[:, kt * P:(kt + 1) * P],
                                                 xht[:, kt, :],
                                                 start=(kt == 0), stop=(kt == nkt - 1))
                            is_rope = ot >= qo - n_rope_ot
                            dest = qt[:, ot, :] if which == "q" else kcache[:, ot, ssl]
                            if not is_rope:
                                nc.scalar.activation(dest, mm_ps[:], AF.Copy)
                            else:
                                raw = miscp.tile([P, SB], BF, tag="raw")
                                nc.scalar.activation(raw[:], mm_ps[:], AF.Copy)
                                ps_sw = psmm.tile([P, SB], FP, tag="mm", name="swp")
                                nc.tensor.matmul(ps_sw[:], rperm_t[:], raw[:],
                                                 start=True, stop=True)
                                m1 = miscp.tile([P, SB], BF, tag="m1")
                                nc.vector.tensor_tensor(m1[:], raw[:], cs_t[:], ALU.mult)
                                m2 = miscp.tile([P, SB], BF, tag="m2")
                                nc.vector.tensor_tensor(m2[:], ps_sw[:], sn_t[:], ALU.mult)
                                nc.vector.tensor_tensor(dest, m1[:], m2[:], ALU.add)
                    # v: kt-outer with sbt pinned accumulators (all 4 psmm slots)
                    v_pss = [psmm.tile([P, dvc], FP, tag="mm", name=f"vps{_i}")
                             for _i in range(sbt)]
                    for kt in range(nkt):
                        wvt = wsmp.tile([P, dvc], BF, tag="wv")
                        nc.sync.dma_start(wvt[:], wv_d[kt])
                        for sc in range(sbt):
                            nc.tensor.matmul(
                                v_pss[sc][:], xht[:, kt, sc * P:(sc + 1) * P], wvt[:],
                                start=(kt == 0), stop=(kt == nkt - 1))
                    for sc in range(sbt):
                        nc.scalar.activation(vcache[:, sb * sbt + sc, :], v_pss[sc][:],
                                             AF.Copy)

                    # --- stage C: attention for q-block sb ---
                    t_max = (sb + 1) * sbt if mask_mode == "causal" else nskt
                    ctxt = ctxp.tile([P, hpc, SB], BF, tag="ctx")
                    for h in range(hpc):
                        rot = qo - n_rope_ot + h // 2
                        rsl = slice(64 * (h % 2), 64 * (h % 2) + 64)
                        sum_ps = psmm.tile([1, SB], FP, tag="mm", name="smps")
                        ctx_ps = psacc.tile([P, SB], FP, tag="actx")
                        pend = []   # es tiles pipelined one step behind the st matmuls
                        for t in range(t_max):
                            tsl = slice(t * P, (t + 1) * P)
                            st_ps = psmm.tile([P, SB], FP, tag="mm", name="stps")
                            has_mask = mask_mode == "full"
                            nc.tensor.matmul(st_ps[:], kcache[:, h, tsl],
                                             qt[:, h, :], start=True, stop=False)
                            nc.tensor.matmul(st_ps[:], kcache[rsl, rot, tsl],
                                             qt[rsl, rot, :],
                                             start=False, stop=not has_mask)
                            if has_mask:
                                mt = miscp.tile([P, SB], BF, tag="mask")
                                nc.sync.dma_start(mt[:], mask_d[t, sb])
                                nc.tensor.matmul(st_ps[:], ident[:], mt[:],
                                                 start=False, stop=True)
                            es = esp.tile([P, SB], BF, tag="es")
                            nc.scalar.activation(es[:], st_ps[:], AF.Exp)
                            if mask_mode == "causal" and t >= sb * sbt:
                                nc.vector.tensor_tensor(
                                    es[:], es[:], tri_t[:, t - sb * sbt, :], ALU.mult)
                            pend.append((t, es))
                            if len(pend) > 1:
                                tp, ep = pend.pop(0)
                                nc.tensor.matmul(sum_ps[:], ones_col[:], ep[:],
                                                 start=(tp == 0), stop=False)
                                nc.tensor.matmul(ctx_ps[:], vcache[:, tp, h * P:(h + 1) * P],
                                                 ep[:], start=(tp == 0), stop=False)
                        tp, ep = pend.pop(0)
                        nc.tensor.matmul(sum_ps[:], ones_col[:], ep[:],
                                         start=(tp == 0), stop=True)
                        nc.tensor.matmul(ctx_ps[:], vcache[:, tp, h * P:(h + 1) * P],
                                         ep[:], start=(tp == 0), stop=True)
                        rec = statp.tile([1, SB], FP, tag="rec")
                        nc.vector.reciprocal(rec[:], sum_ps[:1, :])
                        rb_sb = bcast_sb(rec[:], "rb")
                        nc.vector.tensor_tensor(ctxt[:, h, :], ctx_ps[:], rb_sb[:], ALU.mult)

                    # --- stage D: partial o_proj -> ar_in; AllReduce ---
                    for hc in range(nkt):
                        wot = wsmp.tile([P, ndvt * P], BF, tag="wo")
                        nc.sync.dma_start(wot[:], wo_d[hc])
                        o_ps = psmm.tile([P, SB], FP, tag="mm", name="ops")
                        for dvt in range(ndvt):
                            nc.tensor.matmul(o_ps[:], wot[:, dvt * P:(dvt + 1) * P],
                                             ctxt[:, dvt, :],
                                             start=(dvt == 0), stop=(dvt == ndvt - 1))
                        ao = miscp.tile([P, SB], BF, tag="ao", bufs=3)
                        nc.scalar.activation(ao[:], o_ps[:], AF.Copy)
                        nc.sync.dma_start(ar_in[sb, :, hc, :], ao[:])
                    nc.gpsimd.collective_compute(
                        "AllReduce", ALU.add,
                        replica_groups=[list(range(n_cores))],
                        ins=[ar_in[sb].opt()], outs=[ar_out[sb].opt()])

            # ---------------- stage E + FFN block ----------------
            with ExitStack() as _fstk:
                h2p = _fstk.enter_context(tc.tile_pool(name="h2p", bufs=1))
                utp = _fstk.enter_context(tc.tile_pool(name="utp", bufs=1))
                w1p = _fstk.enter_context(tc.tile_pool(name="w1p", bufs=2))
                w2p = _fstk.enter_context(tc.tile_pool(name="w2p", bufs=2))
                xep = _fstk.enter_context(tc.tile_pool(name="xe", bufs=3))
                x2rp = _fstk.enter_context(tc.tile_pool(name="x2r", bufs=3))
                arrp = _fstk.enter_context(tc.tile_pool(name="arr", bufs=3))
                x2wp = _fstk.enter_context(tc.tile_pool(name="x2w", bufs=3))
                ytp = _fstk.enter_context(tc.tile_pool(name="yt", bufs=2))
                sqep = _fstk.enter_context(tc.tile_pool(name="sqe", bufs=2))
                statep = _fstk.enter_context(tc.tile_pool(name="state", bufs=1))
                absep = _fstk.enter_context(tc.tile_pool(name="abse", bufs=2))
                psf = _fstk.enter_context(tc.tile_pool(name="psf", bufs=4, space="PSUM"))
                def ln_stats_e(sum_ps, ssq_ps):
                    mu = statep.tile([1, SB], FP, tag="mu")
                    nc.scalar.activation(mu[:], sum_ps[:1, :], AF.Copy, scale=1.0 / hid)
                    msq = statep.tile([1, SB], FP, tag="msq")
                    nc.scalar.activation(msq[:], ssq_ps[:1, :], AF.Copy, scale=1.0 / hid)
                    var = statep.tile([1, SB], FP, tag="var")
                    nc.vector.tensor_tensor(var[:], mu[:], mu[:], ALU.mult)
                    nc.vector.tensor_tensor(var[:], msq[:], var[:], ALU.subtract)
                    std = statep.tile([1, SB], FP, tag="std")
                    nc.scalar.activation(std[:], var[:], AF.Sqrt, bias=eps_t[:])
                    rstd = statep.tile([1, SB], FP, tag="rstd")
                    nc.vector.reciprocal(rstd[:], std[:])
                    nmr = statep.tile([1, SB], FP, tag="nmr")
                    nc.vector.tensor_tensor(nmr[:], mu[:], rstd[:], ALU.mult)
                    nc.vector.tensor_scalar_mul(nmr[:], nmr[:], -1.0)
                    return rstd, nmr

                def bcast_e(row, tag):
                    ps = psf.tile([P, SB], FP, tag="ps", name="bc")
                    nc.tensor.matmul(ps[:], ones_row[:], row, start=True, stop=True)
                    sb_t = absep.tile([P, SB], BF, tag=tag, name=tag)
                    nc.scalar.activation(sb_t[:], ps[:], AF.Copy)
                    return sb_t

                def stage_e(sb):
                    ssl = slice(sb * SB, (sb + 1) * SB)
                    sum_ps = psln.tile([1, SB], FP, tag="lsum")
                    ssq_ps = psln.tile([1, SB], FP, tag="lssq")
                    for kt in range(nkt):
                        xtile = xep.tile([P, SB], FP, tag="xe")
                        nc.sync.dma_start(xtile[:], xt_d[kt * P:(kt + 1) * P, ssl])
                        artile = arrp.tile([P, SB], BF, tag="ar")
                        nc.sync.dma_start(artile[:], ar_out[sb, :, kt, :])
                        x2tile = x2wp.tile([P, SB], BF, tag="x2w")
                        nc.vector.tensor_tensor(x2tile[:], xtile[:], artile[:], ALU.add)
                        nc.sync.dma_start(x2t_d[kt, :, ssl], x2tile[:])
                        sq = sqep.tile([P, SB], BF, tag="sq")
                        nc.vector.tensor_tensor(sq[:], x2tile[:], x2tile[:], ALU.mult)
                        nc.tensor.matmul(sum_ps[:], ones_col[:], x2tile[:],
                                         start=(kt == 0), stop=(kt == nkt - 1))
                        nc.tensor.matmul(ssq_ps[:], ones_col[:], sq[:],
                                         start=(kt == 0), stop=(kt == nkt - 1))
                    rstd, nmr = ln_stats_e(sum_ps, ssq_ps)
                    nc.vector.tensor_copy(out=a2_all[:, ssl], in_=rstd[:])
                    nc.vector.tensor_copy(out=c2_all[:, ssl], in_=nmr[:])

                def ffn_block(fb):
                    fsl = slice(fb * FB, (fb + 1) * FB)
                    ab_sbs, cb_sbs = [], []
                    for sub in range(nsub):
                        st = slice(fb * FB + sub * SB, fb * FB + (sub + 1) * SB)
                        ab_sbs.append(bcast_e(a2_all[:, st], "ab"))
                        cb_sbs.append(bcast_e(c2_all[:, st], "cb"))
                    h2 = h2p.tile([P, nkt, FB], BF, tag="h2")
                    for kt in range(nkt):
                        x2tile = x2rp.tile([P, FB], BF, tag="x2l")
                        nc.sync.dma_start(x2tile[:], x2t_d[kt, :, fsl])
                        for sub in range(nsub):
                            dsl = slice(sub * SB, (sub + 1) * SB)
                            nc.vector.tensor_tensor(h2[:, kt, dsl], x2tile[:, dsl],
                                                    ab_sbs[sub][:], ALU.mult)
                            nc.vector.tensor_tensor(h2[:, kt, dsl], h2[:, kt, dsl],
                                                    cb_sbs[sub][:], ALU.add)
                            if ln2_affine:
                                nc.vector.tensor_scalar(
                                    h2[:, kt, dsl], h2[:, kt, dsl],
                                    ln2_wb[:, kt:kt + 1], ln2_wb[:, nkt + kt:nkt + kt + 1],
                                    ALU.mult, ALU.add)
                    ut = utp.tile([P, nft, FB], BF, tag="ut")
                    for ft in range(nft):
                        w1sb = w1p.tile([P, nkt * P], BF, tag="w1")
                        nc.sync.dma_start(w1sb[:], w1_d[ft])
                        u_pss = [psf.tile([P, SB], FP, tag="ps", name=f"ups{_i}")
                                 for _i in range(nsub)]
                        for kt in range(nkt):
                            for sub in range(nsub):
                                nc.tensor.matmul(
                                    u_pss[sub][:], w1sb[:, kt * P:(kt + 1) * P],
                                    h2[:, kt, sub * SB:(sub + 1) * SB],
                                    start=(kt == 0), stop=(kt == nkt - 1))
                        for sub in range(nsub):
                            nc.scalar.activation(ut[:, ft, sub * SB:(sub + 1) * SB],
                                                 u_pss[sub][:], AF.Silu)
                    for hc in range(nkt):
                        w2sb = w2p.tile([P, nft * P], BF, tag="w2")
                        nc.sync.dma_start(w2sb[:], w2_d[hc])
                        y_pss = [psf.tile([P, SB], FP, tag="ps", name=f"yps{_i}")
                                 for _i in range(nsub)]
                        for ft in range(nft):
                            for sub in range(nsub):
                                nc.tensor.matmul(
                                    y_pss[sub][:], w2sb[:, ft * P:(ft + 1) * P],
                                    ut[:, ft, sub * SB:(sub + 1) * SB],
                                    start=(ft == 0), stop=(ft == nft - 1))
                        x2tile = x2rp.tile([P, FB], BF, tag="x2r")
                        nc.sync.dma_start(x2tile[:], x2t_d[hc, :, fsl])
                        yt = ytp.tile([P, FB], FP, tag="yt")
                        for sub in range(nsub):
                            dsl = slice(sub * SB, (sub + 1) * SB)
                            nc.vector.tensor_scalar_mul(yt[:, dsl], x2tile[:, dsl],
                                                        1.0 / n_cores)
                            nc.vector.tensor_tensor(yt[:, dsl], y_pss[sub][:],
                                                    yt[:, dsl], ALU.add)
                        nc.sync.dma_start(y_d[hc, :, fsl], yt[:])

                stage_e(0)
                stage_e(1)
                ffn_block(0)
                stage_e(2)
                stage_e(3)
                ffn_block(1)

    _lp.__exit__(None, None, None)
    nc.compile()
    return nc


# ---------------------------------------------------------------------------
# host side
# ---------------------------------------------------------------------------

def _slab(wT, n_out, n_k):
    """[n_k*P, n_out*P] (contraction-major) -> [n_out, P, n_k*P] slab layout."""
    return np.ascontiguousarray(
        wT.reshape(n_k, P, n_out, P).transpose(2, 1, 0, 3).reshape(n_out, P, n_k * P)
    ).astype(ml_dtypes.bfloat16)


def make_core_inputs(inputs, cfg, mask_mode, ln1_affine, ln2_affine):
    seq, hid, ffn = cfg["seq"], cfg["hid"], cfg["ffn"]
    n_cores, n_heads = cfg["n_cores"], cfg["n_heads"]
    d_nope, d_rope, d_v = cfg["d_nope"], cfg["d_rope"], cfg["d_v"]
    SB = cfg["sb"]
    half = d_rope // 2
    hpc = n_heads // n_cores
    nkt = hid // P
    nsb = seq // SB
    sbt = SB // P
    nskt = seq // P
    n_rope_ot = hpc * d_rope // P
    qo = hpc * d_nope // P + n_rope_ot
    dvc = hpc * d_v
    ndvt = dvc // P
    fpc = ffn // n_cores
    nft = fpc // P

    f32 = np.float32
    bf16 = ml_dtypes.bfloat16
    x = np.asarray(inputs["hidden_states"], dtype=f32)[0]        # [seq, hid]
    xt = np.ascontiguousarray(x.T)                                # [hid, seq]

    inv = (1.0 / (10000.0 ** (np.arange(0, d_rope, 2, dtype=f32) / f32(d_rope)))).astype(f32)
    t = np.arange(seq, dtype=f32)
    freqs = t[:, None] * inv[None, :]
    cosT = np.cos(freqs).astype(f32).T                      # [half, seq]
    sinT = np.sin(freqs).astype(f32).T
    cos128 = np.ascontiguousarray(np.tile(cosT, (P // half, 1)))
    sin128 = np.ascontiguousarray(
        np.tile(np.concatenate([-sinT, sinT], axis=0), (P // d_rope, 1)))
    rperm = np.zeros((P, P), dtype=f32)
    for blk in range(P // d_rope):
        b = blk * d_rope
        for i in range(half):
            rperm[b + half + i, b + i] = 1.0
            rperm[b + i, b + half + i] = 1.0

    common = {"xt": xt, "cos_t": cos128, "sin_t": sin128,
              "rperm": rperm.astype(bf16),
              "ones_b": np.ones((P, P), dtype=bf16),
              "ones_f": np.ones((P, P), dtype=f32)}
    if mask_mode == "causal":
        # tri[i, p, c] = 1 iff query-col c >= key-row i*128+p (block-local)
        tri = np.zeros((sbt, P, SB), dtype=f32)
        cc = np.arange(SB)
        for i in range(sbt):
            for p in range(P):
                tri[i, p] = (cc >= i * P + p)
        common["tri_t"] = tri.astype(bf16)
    elif mask_mode == "full":
        mask = np.asarray(inputs["attention_mask"], dtype=f32)[0, 0]
        mT = np.ascontiguousarray(mask.T)
        m = np.empty((nskt, nsb, P, SB), dtype=f32)
        for tt in range(nskt):
            for qb in range(nsb):
                m[tt, qb] = mT[tt * P:(tt + 1) * P, qb * SB:(qb + 1) * SB]
        common["mask_t"] = m.astype(bf16)
        common["ident_d"] = np.eye(P, dtype=bf16)
    if ln1_affine:
        common["ln1_wb"] = np.ascontiguousarray(np.stack(
            [np.asarray(inputs["ln1_w"], f32), np.asarray(inputs["ln1_b"], f32)]
        ).reshape(2, nkt, P).transpose(2, 0, 1).reshape(P, 2 * nkt))
    if ln2_affine:
        common["ln2_wb"] = np.ascontiguousarray(np.stack(
            [np.asarray(inputs["ln2_w"], f32), np.asarray(inputs["ln2_b"], f32)]
        ).reshape(2, nkt, P).transpose(2, 0, 1).reshape(P, 2 * nkt))

    wq = np.asarray(inputs["w_q"], f32)
    wk = np.asarray(inputs["w_k"], f32)
    wv = np.asarray(inputs["w_v"], f32)
    wo = np.asarray(inputs["w_o"], f32)
    w1 = np.asarray(inputs["w1"], f32)
    w2 = np.asarray(inputs["w2"], f32)

    q_nope_scale = 1.0 / math.sqrt(d_nope)
    q_rope_scale = 1.0 / math.sqrt(d_rope)

    in_maps = []
    for c in range(n_cores):
        heads = range(c * hpc, (c + 1) * hpc)
        nope = np.concatenate([wq[g * d_nope:(g + 1) * d_nope] for g in heads])
        rope = np.concatenate(
            [wq[n_heads * d_nope + g * d_rope: n_heads * d_nope + (g + 1) * d_rope]
             for g in heads])
        wq_c = np.concatenate([nope * q_nope_scale, rope * q_rope_scale])
        wq_t = _slab(np.ascontiguousarray(wq_c.T), qo, nkt)
        nope = np.concatenate([wk[g * d_nope:(g + 1) * d_nope] for g in heads])
        rope = np.concatenate(
            [wk[n_heads * d_nope + g * d_rope: n_heads * d_nope + (g + 1) * d_rope]
             for g in heads])
        wk_t = _slab(np.ascontiguousarray(np.concatenate([nope, rope]).T), qo, nkt)
        wv_c = np.concatenate([wv[g * d_v:(g + 1) * d_v] for g in heads])   # [dvc, hid]
        wv_t = np.ascontiguousarray(wv_c.T.reshape(nkt, P, dvc)).astype(bf16)
        wo_c = wo[:, c * hpc * d_v:(c + 1) * hpc * d_v]                      # [hid, dvc]
        wo_t = _slab(np.ascontiguousarray(wo_c.T), nkt, ndvt)
        w1_t = _slab(np.ascontiguousarray(w1[c * fpc:(c + 1) * fpc].T), nft, nkt)
        w2_t = _slab(np.ascontiguousarray(w2[:, c * fpc:(c + 1) * fpc].T), nkt, nft)
        in_maps.append(dict(common, wq_t=wq_t, wk_t=wk_t, wv_t=wv_t, wo_t=wo_t,
                            w1_t=w1_t, w2_t=w2_t))
    return in_maps


def detect_mask_mode(mask, seq):
    if not mask.any():
        return "zero"
    iu = np.triu_indices(seq, 1)
    upper_blocked = bool((mask[iu] <= -1e8).all())
    il = np.tril_indices(seq)
    lower_zero = bool((mask[il] == 0).all())
    if upper_blocked and lower_zero:
        return "causal"
    return "full"


_BUILT = {}


def run_layer(inputs, cfg, trace=False):
    f32 = np.float32
    mask = np.asarray(inputs["attention_mask"], dtype=f32)[0, 0]
    mask_mode = detect_mask_mode(mask, cfg["seq"])
    ln1_affine = not ((np.asarray(inputs["ln1_w"]) == 1).all()
                     and (np.asarray(inputs["ln1_b"]) == 0).all())
    ln2_affine = not ((np.asarray(inputs["ln2_w"]) == 1).all()
                     and (np.asarray(inputs["ln2_b"]) == 0).all())
    key = (tuple(sorted(cfg.items())), mask_mode, ln1_affine, ln2_affine)
    if key not in _BUILT:
        _BUILT[key] = build_layer_kernel(cfg, mask_mode, ln1_affine, ln2_affine)
    nc = _BUILT[key]
    in_maps = make_core_inputs(inputs, cfg, mask_mode, ln1_affine, ln2_affine)
    res = run_bass_kernel_spmd(nc, in_maps, core_ids=list(range(cfg["n_cores"])),
                               trace=trace)
    hid, seq = cfg["hid"], cfg["seq"]
    acc = np.zeros((hid, seq), dtype=np.float64)
    for c in range(cfg["n_cores"]):
        acc += res.results[c]["y_t"].astype(np.float64).reshape(hid, seq)
    out = acc.T.astype(f32)[None]
    return out, res


def kernel(**inputs):
    out, _ = run_layer(inputs, CFG_FULL)
    return out
